# revision 28
# baseline (speedup 1.0000x reference)
import sys

sys.path.insert(0, "/opt/trn_rl_repo")

import numpy as np

import concourse.bass as bass
import concourse.mybir as mybir
from concourse import bacc
from concourse import bass_utils
from concourse.bass import ds, ts
from concourse.masks import make_identity
from concourse.tile import TileContext

B, S, D, H = 64, 512, 1024, 1024
NCORES = 8
BC = B // NCORES          # batch per core = 8
KC = H // 128             # contraction chunks = 8
NC_ = H // 512            # psum free chunks = 2
STEPS_PER_ITER = 16

f32 = mybir.dt.float32
f32r = mybir.dt.float32r
Tanh = mybir.ActivationFunctionType.Tanh

_CACHED = {}

_orig_run_command = bass_utils.run_command


def _run_command_ldwopt(argv, **kw):
    argv = [a.replace("--enable-ldw-opt=false", "--enable-ldw-opt=true")
            if isinstance(a, str) else a for a in argv]
    return _orig_run_command(argv, **kw)


bass_utils.run_command = _run_command_ldwopt


def _install_ntff_hook():
    """The container's antenv lacks axon_hooks; synthesize it so
    run_bass_kernel_spmd(trace=True) can capture NTFF profiles."""
    import types
    if "antenv.axon_hooks" in sys.modules:
        return
    mod = types.ModuleType("antenv.axon_hooks")
    mod._hook = None

    def set_axon_ntff_profile_hook(h):
        mod._hook = h

    def get_axon_ntff_profile_hook():
        return mod._hook

    mod.set_axon_ntff_profile_hook = set_axon_ntff_profile_hook
    mod.get_axon_ntff_profile_hook = get_axon_ntff_profile_hook
    sys.modules["antenv.axon_hooks"] = mod
    try:
        sys.path.insert(0, "/root/.axon_site/trn_agent_boot")
        import trn_boot
        hook = trn_boot._ntff_profile_via_ctypes("/opt/axon/libaxon_pjrt.so")
        if hook is not None:
            mod._hook = hook
    except Exception:
        pass


def _build(seq_len=S):
    nsteps = seq_len
    niters = nsteps // STEPS_PER_ITER
    rows = nsteps * BC

    nc = bacc.Bacc("TRN2", target_bir_lowering=False, debug=False,
                   num_devices=NCORES)

    # ---- DRAM I/O ----
    xT_d = nc.dram_tensor("xT", [KC, 128, nsteps, BC], f32r, kind="ExternalInput")
    wih0_d = nc.dram_tensor("wih0T", [H, H], f32r, kind="ExternalInput")
    whh0_d = nc.dram_tensor("whh0T", [H, H], f32r, kind="ExternalInput")
    wih1_d = nc.dram_tensor("wih1T", [H, H], f32r, kind="ExternalInput")
    whh1_d = nc.dram_tensor("whh1T", [H, H], f32r, kind="ExternalInput")
    bsum0_d = nc.dram_tensor("bsum0", [1, H], f32r, kind="ExternalInput")
    bsum1_d = nc.dram_tensor("bsum1", [1, H], f32r, kind="ExternalInput")
    ones_d = nc.dram_tensor("ones128", [1, 128], f32r, kind="ExternalInput")
    eye8_d = nc.dram_tensor("eye8", [BC, BC], f32r, kind="ExternalInput")
    eye8f_d = nc.dram_tensor("eye8f", [BC, BC], f32, kind="ExternalInput")
    zer_d = nc.dram_tensor("zeros64", [128, KC * BC], f32r, kind="ExternalInput")

    outh1_d = nc.dram_tensor("outh1", [rows, H], f32, kind="ExternalOutput")
    htfin_d = nc.dram_tensor("htfin", [2, 128, KC * BC], f32r, kind="ExternalOutput")

    # ---- DRAM scratch ----
    u0_d = nc.dram_tensor("u0", [rows, H], f32r, kind="Internal")
    u1_d = nc.dram_tensor("u1", [rows, H], f32r, kind="Internal")

    with TileContext(nc) as tc:
        with tc.tile_pool(name="const", bufs=1) as constp, \
             tc.tile_pool(name="wpool", bufs=1) as wpool, \
             tc.tile_pool(name="state", bufs=1) as statep:

            # resident weights [128, KC*H]: chunk c at cols [c*H, (c+1)*H)
            w_sb = {}
            for nm, d_ in (("wih0", wih0_d), ("whh0", whh0_d),
                           ("wih1", wih1_d), ("whh1", whh1_d)):
                t = wpool.tile([128, KC * H], f32r, name=f"w_{nm}")
                nc.sync.dma_start(
                    t.rearrange("p (c j) -> p c j", c=KC),
                    d_[:].rearrange("(c p) j -> p c j", p=128))
                w_sb[nm] = t
            bs_sb = {}
            for nm, d_ in (("bsum0", bsum0_d), ("bsum1", bsum1_d)):
                t = constp.tile([1, H], f32r, name=nm)
                nc.sync.dma_start(t, d_[:])
                bs_sb[nm] = t
            ones_sb = constp.tile([1, 128], f32r, name="ones")
            nc.sync.dma_start(ones_sb, ones_d[:])
            eye8_sb = constp.tile([BC, BC], f32r, name="eye8")
            nc.sync.dma_start(eye8_sb, eye8_d[:])
            ident = constp.tile([BC, BC], f32, name="ident")
            nc.sync.dma_start(ident, eye8f_d[:])

            # ================= bulk phase: u = actsT @ W + bias ============
            def bulk_phase(actT_d, w_key, bias_key, u_out_d):
                w_t = w_sb[w_key]
                bias_t = bs_sb[bias_key]
                with tc.tile_pool(name="bulkin", bufs=3) as binp, \
                     tc.tile_pool(name="bulkout", bufs=3) as boutp, \
                     tc.tile_pool(name="bulkps", bufs=4, space="PSUM") as bps:
                    for m in range(rows // 128):
                        t0 = m * STEPS_PER_ITER
                        xa = binp.tile([128, KC * 128], f32r, name="xa")
                        nc.sync.dma_start(
                            xa.rearrange("p (c t b) -> p c t b", c=KC,
                                         t=STEPS_PER_ITER),
                            actT_d[:, :, t0:t0 + STEPS_PER_ITER, :].rearrange(
                                "c p t b -> p c t b"))
                        for n in range(NC_):
                            pz = bps.tile([128, 512], f32, name="pz")
                            for c in range(KC):
                                nc.tensor.matmul(
                                    pz,
                                    xa[:, c * 128:(c + 1) * 128],
                                    w_t[:, c * H + n * 512:c * H + n * 512 + 512],
                                    start=(c == 0), stop=False)
                            nc.tensor.matmul(
                                pz, ones_sb,
                                bias_t[:, n * 512:n * 512 + 512],
                                start=False, stop=True)
                            ua = boutp.tile([128, 512], f32r, name="ua")
                            nc.vector.tensor_copy(ua, pz)
                            nc.sync.dma_start(
                                u_out_d[ts(m, 128), ds(n * 512, 512)], ua)

            # ============ combined pipelined recurrence (both layers) ======
            # Layer 0 runs chunk i+1 while layer 1 runs chunk i, paired on
            # the PE via disjoint column groups (tile_position col 0 vs 32).
            # Layer 1's input matmul (u1 chunk) is computed from layer 0's
            # staged hT directly in SBUF after each chunk.
            CST = STEPS_PER_ITER * BC
            CG = KC // NC_

            hT0_st = statep.tile([128, KC * BC], f32r, name="hT0_st")
            hT1_st = statep.tile([128, KC * BC], f32r, name="hT1_st")

            def emit_step(s_, row_off, w_t, u_d, tag, stagT, hT_st,
                          pools, stagh_out_d):
                up, sbp, psp, tpsp = pools

                def lhsT_sl(c):
                    if s_ == 0:
                        return hT_st[:, c * BC:(c + 1) * BC]
                    o = c * CST + (s_ - 1) * BC
                    return stagT[:, o:o + BC]

                u_sm = up.tile([BC, H], f32r, name=f"u_sm{tag}")
                nc.sync.dma_start(u_sm, u_d[ds(row_off + s_ * BC, BC), :])
                h_sm = sbp.tile([BC, H], f32, name=f"h_sm{tag}")
                hTp = tpsp.tile([128, KC * BC], f32, name="hTp")
                stagT_v = stagT.rearrange("p (c t b) -> p c t b", c=KC,
                                          t=STEPS_PER_ITER)
                hTp_v = hTp.rearrange("p (c b) -> p c b", c=KC)
                a_sb = sbp.tile([BC, H], f32, name="a_sb")
                pzs = [psp.tile([BC, 512], f32, name="pz")
                       for _ in range(NC_)]
                for c in range(KC):
                    for n in range(NC_):
                        nc.tensor.matmul(
                            pzs[n], lhsT_sl(c),
                            w_t[:, c * H + n * 512:c * H + n * 512 + 512],
                            start=(c == 0), stop=(c == KC - 1))
                for n in range(NC_):
                    nc.vector.tensor_add(
                        a_sb[:, n * 512:n * 512 + 512], pzs[n],
                        u_sm[:, n * 512:n * 512 + 512])
                    nc.scalar.activation(
                        h_sm[:, n * 512:n * 512 + 512],
                        a_sb[:, n * 512:n * 512 + 512], Tanh)
                    for c in range(n * CG, (n + 1) * CG):
                        nc.tensor.transpose(
                            hTp[:, c * BC:(c + 1) * BC],
                            h_sm[:, c * 128:(c + 1) * 128], ident)
                    nc.vector.tensor_copy(
                        stagT_v[:, n * CG:(n + 1) * CG, s_, :],
                        hTp_v[:, n * CG:(n + 1) * CG, :])
                if s_ == STEPS_PER_ITER - 1:
                    nc.vector.tensor_copy(hT_st, hTp)
                if stagh_out_d is not None:
                    nc.sync.dma_start(
                        stagh_out_d[ds(row_off + s_ * BC, BC), :], h_sm)

            def emit_u1_bulk(row_off, stagT, pools):
                boutp, bps = pools
                w_t = w_sb["wih1"]
                bias_t = bs_sb["bsum1"]
                for n in range(NC_):
                    pu = bps.tile([128, 512], f32, name="pu")
                    for c in range(KC):
                        nc.tensor.matmul(
                            pu, stagT[:, c * CST:(c + 1) * CST],
                            w_t[:, c * H + n * 512:c * H + n * 512 + 512],
                            start=(c == 0), stop=False)
                    nc.tensor.matmul(
                        pu, ones_sb, bias_t[:, n * 512:n * 512 + 512],
                        start=False, stop=True)
                    ub = boutp.tile([128, 512], f32r, name="ub")
                    nc.vector.tensor_copy(ub, pu)
                    nc.sync.dma_start(
                        u1_d[ds(row_off, 128), ds(n * 512, 512)], ub)

            bulk_phase(xT_d, "wih0", "bsum0", u0_d)

            nc.sync.dma_start(hT0_st, zer_d[:])
            nc.sync.dma_start(hT1_st, zer_d[:])

            with tc.tile_pool(name="recu", bufs=3) as up, \
                 tc.tile_pool(name="recstag", bufs=1) as stp, \
                 tc.tile_pool(name="recsb", bufs=2) as sbp, \
                 tc.tile_pool(name="recout", bufs=2) as boutp, \
                 tc.tile_pool(name="recps", bufs=5, space="PSUM") as psp, \
                 tc.tile_pool(name="rectps", bufs=2, space="PSUM") as tpsp, \
                 tc.tile_pool(name="recbps", bufs=1, space="PSUM") as bps:
                spools = (up, sbp, psp, tpsp)
                bpools = (boutp, bps)

                # prologue: layer 0 chunk 0, then u1 chunk 0
                stagT0p = stp.tile([128, CST * KC], f32r, name="stagT0p")
                for s_ in range(STEPS_PER_ITER):
                    emit_step(s_, 0, w_sb["whh0"], u0_d, "0", stagT0p,
                              hT0_st, spools, None)
                emit_u1_bulk(0, stagT0p, bpools)

                if niters > 1:
                    with tc.For_i(0, niters - 1, 1,
                                  staggered_reset=True,
                                  hint_engines=(mybir.EngineType.PE,
                                                mybir.EngineType.DVE,
                                                mybir.EngineType.SP)) as it:
                        stagT0 = stp.tile([128, CST * KC], f32r,
                                          name="stagT0")
                        stagT1 = stp.tile([128, CST * KC], f32r,
                                          name="stagT1")
                        for s_ in range(STEPS_PER_ITER):
                            emit_step(s_, it * 128 + 128, w_sb["whh0"],
                                      u0_d, "0", stagT0, hT0_st, spools,
                                      None)
                            emit_step(s_, it * 128, w_sb["whh1"],
                                      u1_d, "1", stagT1, hT1_st, spools,
                                      outh1_d)
                        emit_u1_bulk(it * 128 + 128, stagT0, bpools)

                # epilogue: layer 1 last chunk
                stagT1e = stp.tile([128, CST * KC], f32r, name="stagT1e")
                for s_ in range(STEPS_PER_ITER):
                    emit_step(s_, (niters - 1) * 128, w_sb["whh1"], u1_d,
                              "1", stagT1e, hT1_st, spools, outh1_d)

            nc.sync.dma_start(htfin_d[0], hT0_st)
            nc.sync.dma_start(htfin_d[1], hT1_st)

    nc.finalize()
    return nc


def _prep_inputs(x, W_ih0, b_ih0, W_hh0, b_hh0, W_ih1, b_ih1, W_hh1, b_hh1,
                 seq_len=S):
    shared = {
        "wih0T": np.ascontiguousarray(W_ih0.T),
        "whh0T": np.ascontiguousarray(W_hh0.T),
        "wih1T": np.ascontiguousarray(W_ih1.T),
        "whh1T": np.ascontiguousarray(W_hh1.T),
        "bsum0": np.ascontiguousarray((b_ih0 + b_hh0)[None, :]),
        "bsum1": np.ascontiguousarray((b_ih1 + b_hh1)[None, :]),
        "ones128": np.ones((1, 128), np.float32),
        "eye8": np.eye(BC, dtype=np.float32),
        "eye8f": np.eye(BC, dtype=np.float32),
        "zeros64": np.zeros((128, KC * BC), np.float32),
    }
    in_maps = []
    for c in range(NCORES):
        xs = x[c * BC:(c + 1) * BC, :seq_len]            # [BC, s, D]
        xT = np.ascontiguousarray(
            xs.transpose(2, 1, 0).reshape(KC, 128, seq_len, BC))
        m = dict(shared)
        m["xT"] = xT
        in_maps.append(m)
    return in_maps


def _gather(results, seq_len=S):
    outs = np.empty((B, seq_len, H), np.float32)
    hT = np.empty((2, B, H), np.float32)
    for c in range(NCORES):
        oh = results[c]["outh1"].reshape(seq_len, BC, H)
        outs[c * BC:(c + 1) * BC] = oh.transpose(1, 0, 2)
        hf = results[c]["htfin"].reshape(2, 128, KC, BC)
        hT[:, c * BC:(c + 1) * BC, :] = hf.transpose(0, 3, 2, 1).reshape(
            2, BC, H)
    return outs, hT


def kernel(x, W_ih0, b_ih0, W_hh0, b_hh0, W_ih1, b_ih1, W_hh1, b_hh1,
           seq_len=S, trace=False):
    x = np.asarray(x, np.float32)
    args = [np.asarray(a, np.float32) for a in
            (W_ih0, b_ih0, W_hh0, b_hh0, W_ih1, b_ih1, W_hh1, b_hh1)]
    if trace:
        _install_ntff_hook()
    if seq_len not in _CACHED:
        _CACHED[seq_len] = _build(seq_len)
    nc = _CACHED[seq_len]
    in_maps = _prep_inputs(x, *args, seq_len=seq_len)
    res = bass_utils.run_bass_kernel_spmd(
        nc, in_maps, core_ids=list(range(NCORES)), trace=trace)
    outs, hT = _gather(res.results, seq_len=seq_len)
    kernel.last_result = res
    return outs, hT


# revision 29
# speedup vs baseline: 1.0155x; 1.0155x over previous
import sys

sys.path.insert(0, "/opt/trn_rl_repo")

import numpy as np

import concourse.bass as bass
import concourse.mybir as mybir
from concourse import bacc
from concourse import bass_utils
from concourse.bass import ds, ts
from concourse.masks import make_identity
from concourse.tile import TileContext

B, S, D, H = 64, 512, 1024, 1024
NCORES = 8
BC = B // NCORES          # batch per core = 8
KC = H // 128             # contraction chunks = 8
NC_ = H // 512            # psum free chunks = 2
STEPS_PER_ITER = 16

f32 = mybir.dt.float32
f32r = mybir.dt.float32r
Tanh = mybir.ActivationFunctionType.Tanh

_CACHED = {}

_orig_run_command = bass_utils.run_command


def _run_command_ldwopt(argv, **kw):
    argv = [a.replace("--enable-ldw-opt=false", "--enable-ldw-opt=true")
            if isinstance(a, str) else a for a in argv]
    return _orig_run_command(argv, **kw)


bass_utils.run_command = _run_command_ldwopt


def _install_ntff_hook():
    """The container's antenv lacks axon_hooks; synthesize it so
    run_bass_kernel_spmd(trace=True) can capture NTFF profiles."""
    import types
    if "antenv.axon_hooks" in sys.modules:
        return
    mod = types.ModuleType("antenv.axon_hooks")
    mod._hook = None

    def set_axon_ntff_profile_hook(h):
        mod._hook = h

    def get_axon_ntff_profile_hook():
        return mod._hook

    mod.set_axon_ntff_profile_hook = set_axon_ntff_profile_hook
    mod.get_axon_ntff_profile_hook = get_axon_ntff_profile_hook
    sys.modules["antenv.axon_hooks"] = mod
    try:
        sys.path.insert(0, "/root/.axon_site/trn_agent_boot")
        import trn_boot
        hook = trn_boot._ntff_profile_via_ctypes("/opt/axon/libaxon_pjrt.so")
        if hook is not None:
            mod._hook = hook
    except Exception:
        pass


def _build(seq_len=S):
    nsteps = seq_len
    niters = nsteps // STEPS_PER_ITER
    rows = nsteps * BC

    nc = bacc.Bacc("TRN2", target_bir_lowering=False, debug=False,
                   num_devices=NCORES)

    # ---- DRAM I/O ----
    xT_d = nc.dram_tensor("xT", [KC, 128, nsteps, BC], f32r, kind="ExternalInput")
    wih0_d = nc.dram_tensor("wih0T", [H, H], f32r, kind="ExternalInput")
    whh0_d = nc.dram_tensor("whh0T", [H, H], f32r, kind="ExternalInput")
    wih1_d = nc.dram_tensor("wih1T", [H, H], f32r, kind="ExternalInput")
    whh1_d = nc.dram_tensor("whh1T", [H, H], f32r, kind="ExternalInput")
    bsum0_d = nc.dram_tensor("bsum0", [1, H], f32r, kind="ExternalInput")
    bsum1_d = nc.dram_tensor("bsum1", [1, H], f32r, kind="ExternalInput")
    ones_d = nc.dram_tensor("ones128", [1, 128], f32r, kind="ExternalInput")
    eye8_d = nc.dram_tensor("eye8", [BC, BC], f32r, kind="ExternalInput")
    eye8f_d = nc.dram_tensor("eye8f", [BC, BC], f32, kind="ExternalInput")
    zer_d = nc.dram_tensor("zeros64", [128, KC * BC], f32r, kind="ExternalInput")

    outh1_d = nc.dram_tensor("outh1", [rows, H], f32, kind="ExternalOutput")
    htfin_d = nc.dram_tensor("htfin", [2, 128, KC * BC], f32r, kind="ExternalOutput")

    # ---- DRAM scratch ----
    u0_d = nc.dram_tensor("u0", [rows, H], f32r, kind="Internal")
    u1_d = nc.dram_tensor("u1", [rows, H], f32r, kind="Internal")

    with TileContext(nc) as tc:
        with tc.tile_pool(name="const", bufs=1) as constp, \
             tc.tile_pool(name="wpool", bufs=1) as wpool, \
             tc.tile_pool(name="state", bufs=1) as statep:

            # resident weights [128, KC*H]: chunk c at cols [c*H, (c+1)*H)
            w_sb = {}
            for nm, d_ in (("wih0", wih0_d), ("whh0", whh0_d),
                           ("wih1", wih1_d), ("whh1", whh1_d)):
                t = wpool.tile([128, KC * H], f32r, name=f"w_{nm}")
                nc.sync.dma_start(
                    t.rearrange("p (c j) -> p c j", c=KC),
                    d_[:].rearrange("(c p) j -> p c j", p=128))
                w_sb[nm] = t
            bs_sb = {}
            for nm, d_ in (("bsum0", bsum0_d), ("bsum1", bsum1_d)):
                t = constp.tile([1, H], f32r, name=nm)
                nc.sync.dma_start(t, d_[:])
                bs_sb[nm] = t
            ones_sb = constp.tile([1, 128], f32r, name="ones")
            nc.sync.dma_start(ones_sb, ones_d[:])
            eye8_sb = constp.tile([BC, BC], f32r, name="eye8")
            nc.sync.dma_start(eye8_sb, eye8_d[:])
            ident = constp.tile([BC, BC], f32, name="ident")
            nc.sync.dma_start(ident, eye8f_d[:])

            # ================= bulk phase: u = actsT @ W + bias ============
            def bulk_phase(actT_d, w_key, bias_key, u_out_d):
                w_t = w_sb[w_key]
                bias_t = bs_sb[bias_key]
                with tc.tile_pool(name="bulkin", bufs=3) as binp, \
                     tc.tile_pool(name="bulkout", bufs=3) as boutp, \
                     tc.tile_pool(name="bulkps", bufs=4, space="PSUM") as bps:
                    for m in range(rows // 128):
                        t0 = m * STEPS_PER_ITER
                        xa = binp.tile([128, KC * 128], f32r, name="xa")
                        nc.sync.dma_start(
                            xa.rearrange("p (c t b) -> p c t b", c=KC,
                                         t=STEPS_PER_ITER),
                            actT_d[:, :, t0:t0 + STEPS_PER_ITER, :].rearrange(
                                "c p t b -> p c t b"))
                        for n in range(NC_):
                            pz = bps.tile([128, 512], f32, name="pz")
                            for c in range(KC):
                                nc.tensor.matmul(
                                    pz,
                                    xa[:, c * 128:(c + 1) * 128],
                                    w_t[:, c * H + n * 512:c * H + n * 512 + 512],
                                    start=(c == 0), stop=False)
                            nc.tensor.matmul(
                                pz, ones_sb,
                                bias_t[:, n * 512:n * 512 + 512],
                                start=False, stop=True)
                            ua = boutp.tile([128, 512], f32r, name="ua")
                            nc.vector.tensor_copy(ua, pz)
                            nc.sync.dma_start(
                                u_out_d[ts(m, 128), ds(n * 512, 512)], ua)

            # ============ combined pipelined recurrence (both layers) ======
            # Layer 0 runs chunk i+1 while layer 1 runs chunk i, paired on
            # the PE via disjoint column groups (tile_position col 0 vs 32).
            # Layer 1's input matmul (u1 chunk) is computed from layer 0's
            # staged hT directly in SBUF after each chunk.
            CST = STEPS_PER_ITER * BC
            CG = KC // NC_

            hT0_st = statep.tile([128, KC * BC], f32r, name="hT0_st")
            hT1_st = statep.tile([128, KC * BC], f32r, name="hT1_st")

            def emit_step(s_, row_off, w_t, u_d, tag, stagT, hT_st,
                          pools, stagh_out_d):
                up, sbp, psp, tpsp = pools

                def lhsT_sl(c):
                    if s_ == 0:
                        return hT_st[:, c * BC:(c + 1) * BC]
                    o = c * CST + (s_ - 1) * BC
                    return stagT[:, o:o + BC]

                u_sm = up.tile([BC, H], f32r, name=f"u_sm{tag}")
                nc.sync.dma_start(u_sm, u_d[ds(row_off + s_ * BC, BC), :])
                h_sm = sbp.tile([BC, H], f32, name=f"h_sm{tag}")
                hTp = tpsp.tile([128, KC * BC], f32, name="hTp")
                stagT_v = stagT.rearrange("p (c t b) -> p c t b", c=KC,
                                          t=STEPS_PER_ITER)
                hTp_v = hTp.rearrange("p (c b) -> p c b", c=KC)
                a_sb = sbp.tile([BC, H], f32, name="a_sb")
                pzs = [psp.tile([BC, 512], f32, name="pz")
                       for _ in range(NC_)]
                for c in range(KC):
                    for n in range(NC_):
                        nc.tensor.matmul(
                            pzs[n], lhsT_sl(c),
                            w_t[:, c * H + n * 512:c * H + n * 512 + 512],
                            start=(c == 0), stop=(c == KC - 1))
                for n in range(NC_):
                    nc.vector.tensor_add(
                        a_sb[:, n * 512:n * 512 + 512], pzs[n],
                        u_sm[:, n * 512:n * 512 + 512])
                    nc.scalar.activation(
                        h_sm[:, n * 512:n * 512 + 512],
                        a_sb[:, n * 512:n * 512 + 512], Tanh)
                    for c in range(n * CG, (n + 1) * CG):
                        nc.tensor.transpose(
                            hTp[:, c * BC:(c + 1) * BC],
                            h_sm[:, c * 128:(c + 1) * 128], ident)
                    nc.vector.tensor_copy(
                        stagT_v[:, n * CG:(n + 1) * CG, s_, :],
                        hTp_v[:, n * CG:(n + 1) * CG, :])
                if s_ == STEPS_PER_ITER - 1:
                    nc.vector.tensor_copy(hT_st, hTp)
                if stagh_out_d is not None:
                    nc.sync.dma_start(
                        stagh_out_d[ds(row_off + s_ * BC, BC), :], h_sm)

            def emit_u1_bulk(row_off, stagT, pools):
                boutp, bps = pools
                w_t = w_sb["wih1"]
                bias_t = bs_sb["bsum1"]
                for n in range(NC_):
                    pu = bps.tile([128, 512], f32, name="pu")
                    for c in range(KC):
                        nc.tensor.matmul(
                            pu, stagT[:, c * CST:(c + 1) * CST],
                            w_t[:, c * H + n * 512:c * H + n * 512 + 512],
                            start=(c == 0), stop=False)
                    nc.tensor.matmul(
                        pu, ones_sb, bias_t[:, n * 512:n * 512 + 512],
                        start=False, stop=True)
                    ub = boutp.tile([128, 512], f32r, name="ub")
                    nc.vector.tensor_copy(ub, pu)
                    nc.sync.dma_start(
                        u1_d[ds(row_off, 128), ds(n * 512, 512)], ub)

            nc.sync.dma_start(hT0_st, zer_d[:])
            nc.sync.dma_start(hT1_st, zer_d[:])

            with tc.tile_pool(name="recu", bufs=2) as up, \
                 tc.tile_pool(name="bulkin", bufs=2) as binp, \
                 tc.tile_pool(name="recstag", bufs=1) as stp, \
                 tc.tile_pool(name="recsb", bufs=2) as sbp, \
                 tc.tile_pool(name="recout", bufs=2) as boutp, \
                 tc.tile_pool(name="recps", bufs=5, space="PSUM") as psp, \
                 tc.tile_pool(name="rectps", bufs=2, space="PSUM") as tpsp, \
                 tc.tile_pool(name="recbps", bufs=1, space="PSUM") as bps:
                spools = (up, sbp, psp, tpsp)
                bpools = (boutp, bps)
                w_ih0 = w_sb["wih0"]
                b0_t = bs_sb["bsum0"]

                def emit_bulk_m(m):
                    t0 = m * STEPS_PER_ITER
                    xa = binp.tile([128, KC * 128], f32r, name="xa")
                    nc.sync.dma_start(
                        xa.rearrange("p (c t b) -> p c t b", c=KC,
                                     t=STEPS_PER_ITER),
                        xT_d[:, :, t0:t0 + STEPS_PER_ITER, :].rearrange(
                            "c p t b -> p c t b"))
                    for n in range(NC_):
                        pa = bps.tile([128, 512], f32, name="pu")
                        for c in range(KC):
                            nc.tensor.matmul(
                                pa, xa[:, c * 128:(c + 1) * 128],
                                w_ih0[:, c * H + n * 512:c * H + n * 512 + 512],
                                start=(c == 0), stop=False)
                        nc.tensor.matmul(
                            pa, ones_sb, b0_t[:, n * 512:n * 512 + 512],
                            start=False, stop=True)
                        ux = boutp.tile([128, 512], f32r, name="ub")
                        nc.vector.tensor_copy(ux, pa)
                        nc.sync.dma_start(
                            u0_d[ts(m, 128), ds(n * 512, 512)], ux)

                # phase A m-tile 0, then prologue steps interleaved with
                # the remaining phase-A tiles
                emit_bulk_m(0)
                stagT0p = stp.tile([128, CST * KC], f32r, name="stagT0p")
                for s_ in range(STEPS_PER_ITER):
                    emit_step(s_, 0, w_sb["whh0"], u0_d, "0", stagT0p,
                              hT0_st, spools, None)
                    if s_ + 1 < rows // 128:
                        emit_bulk_m(s_ + 1)
                for m in range(STEPS_PER_ITER + 1, rows // 128):
                    emit_bulk_m(m)
                emit_u1_bulk(0, stagT0p, bpools)

                if niters > 1:
                    with tc.For_i(0, niters - 1, 1,
                                  staggered_reset=True,
                                  hint_engines=(mybir.EngineType.PE,
                                                mybir.EngineType.DVE,
                                                mybir.EngineType.SP)) as it:
                        stagT0 = stp.tile([128, CST * KC], f32r,
                                          name="stagT0")
                        stagT1 = stp.tile([128, CST * KC], f32r,
                                          name="stagT1")
                        for s_ in range(STEPS_PER_ITER):
                            emit_step(s_, it * 128 + 128, w_sb["whh0"],
                                      u0_d, "0", stagT0, hT0_st, spools,
                                      None)
                            emit_step(s_, it * 128, w_sb["whh1"],
                                      u1_d, "1", stagT1, hT1_st, spools,
                                      outh1_d)
                        emit_u1_bulk(it * 128 + 128, stagT0, bpools)

                # epilogue: layer 1 last chunk
                stagT1e = stp.tile([128, CST * KC], f32r, name="stagT1e")
                for s_ in range(STEPS_PER_ITER):
                    emit_step(s_, (niters - 1) * 128, w_sb["whh1"], u1_d,
                              "1", stagT1e, hT1_st, spools, outh1_d)

            nc.sync.dma_start(htfin_d[0], hT0_st)
            nc.sync.dma_start(htfin_d[1], hT1_st)

    nc.finalize()
    return nc


def _prep_inputs(x, W_ih0, b_ih0, W_hh0, b_hh0, W_ih1, b_ih1, W_hh1, b_hh1,
                 seq_len=S):
    shared = {
        "wih0T": np.ascontiguousarray(W_ih0.T),
        "whh0T": np.ascontiguousarray(W_hh0.T),
        "wih1T": np.ascontiguousarray(W_ih1.T),
        "whh1T": np.ascontiguousarray(W_hh1.T),
        "bsum0": np.ascontiguousarray((b_ih0 + b_hh0)[None, :]),
        "bsum1": np.ascontiguousarray((b_ih1 + b_hh1)[None, :]),
        "ones128": np.ones((1, 128), np.float32),
        "eye8": np.eye(BC, dtype=np.float32),
        "eye8f": np.eye(BC, dtype=np.float32),
        "zeros64": np.zeros((128, KC * BC), np.float32),
    }
    in_maps = []
    for c in range(NCORES):
        xs = x[c * BC:(c + 1) * BC, :seq_len]            # [BC, s, D]
        xT = np.ascontiguousarray(
            xs.transpose(2, 1, 0).reshape(KC, 128, seq_len, BC))
        m = dict(shared)
        m["xT"] = xT
        in_maps.append(m)
    return in_maps


def _gather(results, seq_len=S):
    outs = np.empty((B, seq_len, H), np.float32)
    hT = np.empty((2, B, H), np.float32)
    for c in range(NCORES):
        oh = results[c]["outh1"].reshape(seq_len, BC, H)
        outs[c * BC:(c + 1) * BC] = oh.transpose(1, 0, 2)
        hf = results[c]["htfin"].reshape(2, 128, KC, BC)
        hT[:, c * BC:(c + 1) * BC, :] = hf.transpose(0, 3, 2, 1).reshape(
            2, BC, H)
    return outs, hT


def kernel(x, W_ih0, b_ih0, W_hh0, b_hh0, W_ih1, b_ih1, W_hh1, b_hh1,
           seq_len=S, trace=False):
    x = np.asarray(x, np.float32)
    args = [np.asarray(a, np.float32) for a in
            (W_ih0, b_ih0, W_hh0, b_hh0, W_ih1, b_ih1, W_hh1, b_hh1)]
    if trace:
        _install_ntff_hook()
    if seq_len not in _CACHED:
        _CACHED[seq_len] = _build(seq_len)
    nc = _CACHED[seq_len]
    in_maps = _prep_inputs(x, *args, seq_len=seq_len)
    res = bass_utils.run_bass_kernel_spmd(
        nc, in_maps, core_ids=list(range(NCORES)), trace=trace)
    outs, hT = _gather(res.results, seq_len=seq_len)
    kernel.last_result = res
    return outs, hT
